# revision 11
# baseline (speedup 1.0000x reference)
"""Trainium2 Bass kernel for nn_EquilibriumModel (gnn_message_passing).

Strategy
--------
The model is a 16-step sequential equilibrium scan over a 16x256 "trail grid"
graph.  The heavy inputs (connectivity / incidence, each [7920, 4096]) are the
signed incidence matrices of a fixed, deterministic topology: trail edges
(i,t)->(i+1,t) and deviation edges (i,t)->(i,t+1), with `sequences` equal to
arange.  kernel() verifies that topology on the host (falling back to a pure
numpy port of the reference if anything differs) and then runs a specialized
on-chip program:

  - partition dim = t (mod 128), free dim = (step, half, xyz-comp)
  - the per-step deviation-force stencil (t -> t+-1 shifts) is done on the
    TensorEngine with constant +-1 shift matrices accumulated in PSUM
    (compute engines cannot address partition offsets != 0/32/64/96)
  - sum-of-squares + sqrt via ScalarEngine Square(accum_out)/Sqrt, the rest of
    the per-step chain (normalize, plane intersection, position update) on the
    VectorEngine
  - edge lengths / trail forces are produced in bulk from the accumulated
    position history; host code only re-lays-out device blocks and passes
    through the outputs the reference returns verbatim (loads, dev-edge
    forces, zero rows of reactions).

The same program is replicated SPMD on all 8 NeuronCores (the sequential
scan does not shard without paying a ~10us collective floor per step, which
would dominate); core 0's outputs are used.
"""

import sys

import numpy as np

for _p in ("/opt/trn_rl_repo",):
    if _p not in sys.path:
        sys.path.insert(0, _p)

import concourse.bass as bass  # noqa: E402
import concourse.tile as tile  # noqa: E402
from concourse import mybir  # noqa: E402
from concourse.bass_utils import run_bass_kernel_spmd  # noqa: E402

F32 = mybir.dt.float32
Alu = mybir.AluOpType
Act = mybir.ActivationFunctionType
AX = mybir.AxisListType

N_TRAILS = 256
N_STEPS = 16
N_NODES = N_TRAILS * N_STEPS  # 4096
N_TRAIL_E = (N_STEPS - 1) * N_TRAILS  # 3840
N_DEV_E = N_STEPS * (N_TRAILS - 1)  # 4080
N_EDGES = N_TRAIL_E + N_DEV_E  # 7920
P = 128
H = 2  # halves of the 256 trails on the 128 partitions: t = h*128 + p

IN_NAMES = [
    "xyz0_blk",  # [128, 6]    step-0 positions, (p, h, c)
    "loads_blk",  # [128, 96]  (p, i, h, c)
    "ori_blk",  # [128, 96]
    "nor_blk",  # [128, 96]
    "len0_blk",  # [128, 32]   (p, i, h)
    "fdev_blk",  # [128, 32]   deviation-edge forces, t=255 slot zeroed
    "difu_m",  # [128, 128]  out[m] = x[m+1] - x[m]
    "b0_m",  # [128, 128]     out[127] += rhs[0]
    "difd_m",  # [128, 128]   out[m] = s[m] - s[m-1]
    "b127n_m",  # [128, 128]  out[0] += -rhs[127]
]
OUT_NAMES = [
    "out_xyz",  # [128, 96]   (p, i, h, c)
    "out_res",  # [128, 6]    residuals after step 15, (p, h, c)
    "out_dlen",  # [128, 32]  deviation-edge lengths (p, i, h)
    "out_tlen",  # [128, 30]  trail-edge lengths (p, i<15, h)
    "out_tf",  # [128, 30]    trail forces (p, i<15, h)
]
IN_SHAPES = {
    "xyz0_blk": (P, 6),
    "loads_blk": (P, 96),
    "ori_blk": (P, 96),
    "nor_blk": (P, 96),
    "len0_blk": (P, 32),
    "fdev_blk": (P, 32),
    "difu_m": (P, P),
    "b0_m": (P, P),
    "difd_m": (P, P),
    "b127n_m": (P, P),
}
OUT_SHAPES = {
    "out_xyz": (P, 96),
    "out_res": (P, 6),
    "out_dlen": (P, 32),
    "out_tlen": (P, 30),
    "out_tf": (P, 30),
}

TINY = 1e-38  # < ulp of any ss we care about; turns sqrt(0)=0 into 1e-19


def _emit(nc, ins: dict, outs: dict):
    """Emit the whole program in raw bass (no Tile scheduler).

    This walrus build rejects any DVE-family (S3S3D3) instruction carrying
    more than ONE semaphore wait, which Tile's scheduler freely emits.  The
    dependency structure here is a simple 3-engine pipeline, so semaphores
    are placed by hand:

      - sem D: every input DMA incs it by 16 (one wait, value 160, observes
        ALL input loads at once)
      - sem T/A/V: progress counters on PE / ACT / DVE, bumped at the few
        spots another engine waits on
      - per-engine program order covers everything else

    Every DVE instruction ends up with <= 1 wait; Matmult carries up to 2
    (this walrus accepts that); output DMAs wait on one final DVE join.

    Per-step semaphore targets (1-based):
      PE:  T_dp(i) = 2i+1 (Dp ready),  T_dev(i) = 2i+2 (DEVp ready)
      ACT: A_dl(i) = 2i+1 (sqrt ss_d), A_nr(i) = 2i+2 (sqrt ss_r, i<15)
      DVE: V=1 after prologue memsets, then per step:
           V_svec(i) = 3i+2, V_rnew(i) = 3i+3, V_xnext(i) = 3i+4 (i<15)
           step 15: V_svec=47, V_rnew=48; epilogue: V_tss=49, V_tf=50,
           V_fin=51 (join on ACT's last write)
    """
    from contextlib import ExitStack

    with ExitStack() as ctx:

        def t(tag, shape):
            return ctx.enter_context(nc.sbuf_tensor(tag, list(shape), F32))

        # persistent state/history
        XYZ = t("XYZ", (P, 96))
        RES = t("RES", (P, 102))  # 17 slots of (h,c); slot 0 = zeros
        LEN = t("LEN", (P, 30))
        NRM = t("NRM", (P, 30))
        DLEN = t("DLEN", (P, 32))
        # inputs
        LOADS = t("LOADS", (P, 96))
        ORI = t("ORI", (P, 96))
        NOR = t("NOR", (P, 96))
        LEN0 = t("LEN0", (P, 32))
        FDEV = t("FDEV", (P, 32))
        DIFU = t("DIFU", (P, P))
        B0 = t("B0", (P, P))
        DIFD = t("DIFD", (P, P))
        B127N = t("B127N", (P, P))
        # derived statics
        NSAFE = t("NSAFE", (P, 96))
        NMASK = t("NMASK", (P, 96))
        L0Z = t("L0Z", (P, 32))
        ZA = t("ZA", (P, 32))
        ZN = t("ZN", (P, 32))
        ZNB = t("ZNB", (P, 96))
        ONE96 = t("ONE96", (P, 96))
        ZERO96 = t("ZERO96", (P, 96))
        ONES2 = t("ONES2", (P, 2))
        ZERO2 = t("ZERO2", (P, 2))
        BIAS1 = t("BIAS1", (P, 1))
        BIAS0 = t("BIAS0", (P, 1))
        # per-step scratch
        SQS = t("SQS", (P, 6))
        SSD = t("SSD", (P, 2))
        RINV = t("RINV", (P, 2))
        W = t("W", (P, 6))
        SVEC = t("SVEC", (P, 6))
        R1 = t("R1", (P, 6))
        MABS = t("MABS", (P, 2))
        M = t("M", (P, 2))
        SSR = t("SSR", (P, 2))
        RINVR = t("RINVR", (P, 2))
        UNIT = t("UNIT", (P, 6))
        NU = t("NU", (P, 6))
        DEN = t("DEN", (P, 2))
        RDEN = t("RDEN", (P, 2))
        OMX = t("OMX", (P, 6))
        CX = t("CX", (P, 6))
        CZ = t("CZ", (P, 2))
        LP = t("LP", (P, 2))
        LT = t("LT", (P, 2))
        XT = t("XT", (P, 6))
        # epilogue scratch
        TD = t("TD", (P, 90))
        TSQ = t("TSQ", (P, 90))
        TSS = t("TSS", (P, 30))
        TLEN = t("TLEN", (P, 30))
        MNEG = t("MNEG", (P, 30))
        SGN = t("SGN", (P, 30))
        TF = t("TF", (P, 30))

        JNK = t("JNK", (P, 1))

        Dp = ctx.enter_context(nc.psum_tensor("Dp", [P, 6], F32))
        DEVp = ctx.enter_context(nc.psum_tensor("DEVp", [P, 6], F32))

        dma_i = nc.alloc_semaphore("dma_in")
        dma_o = nc.alloc_semaphore("dma_out")
        sv = nc.alloc_semaphore("sv")  # DVE progress
        sa = nc.alloc_semaphore("sa")  # ACT progress
        st = nc.alloc_semaphore("st")  # PE progress

        def g3(T, off6):  # [128, 2, 3] view at (h,c) group offset
            return T[:, off6 : off6 + 6].rearrange("p (h c) -> p h c", h=2)

        # semaphore targets (see docstring)
        def v_svec(i):
            return 3 * i + 2

        def v_rnew(i):
            return 3 * i + 3

        def v_xnext(i):
            return 3 * i + 4

        V_TSS = v_rnew(15) + 1  # 49
        V_TF = V_TSS + 1
        V_FIN = V_TF + 1
        A_TLEN = 2 * 15 + 1 + 1  # 32

        v = nc.vector
        s = nc.scalar

        # The engines are deep pipelines: a dependent op on the SAME engine
        # needs an explicit DRAIN between write and read (Tile inserts these
        # automatically; raw bass must do it by hand).  Cross-engine signal
        # increments ride the drain so the data is visible when the sem fires.
        def vd(inc=None):
            d = v.drain()
            if inc is not None:
                d.then_inc(inc)

        def sd(inc=None):
            d = s.drain()
            if inc is not None:
                d.then_inc(inc)

        with nc.Block() as block:

            @block.sync
            def _(sync):
                for name, dst in [
                    ("xyz0_blk", XYZ[:, 0:6]),
                    ("loads_blk", LOADS[:]),
                    ("ori_blk", ORI[:]),
                    ("nor_blk", NOR[:]),
                    ("len0_blk", LEN0[:]),
                    ("fdev_blk", FDEV[:]),
                    ("difu_m", DIFU[:]),
                    ("b0_m", B0[:]),
                    ("difd_m", DIFD[:]),
                    ("b127n_m", B127N[:]),
                ]:
                    nc.sync.dma_start(dst, ins[name][:]).then_inc(dma_i, 16)
                # outputs: the first store waits on the final DVE join; the
                # rest follow in sequencer program order.
                nc.sync.wait_ge(sv, V_FIN)
                nc.sync.dma_start(outs["out_xyz"][:], XYZ[:]).then_inc(dma_o, 16)
                nc.sync.dma_start(outs["out_res"][:], RES[:, 96:102]).then_inc(dma_o, 16)
                nc.sync.dma_start(outs["out_dlen"][:], DLEN[:]).then_inc(dma_o, 16)
                nc.sync.dma_start(outs["out_tlen"][:], TLEN[:]).then_inc(dma_o, 16)
                nc.sync.dma_start(outs["out_tf"][:], TF[:]).then_inc(dma_o, 16)
                nc.sync.wait_ge(dma_o, 80)

            @block.tensor
            def _(tensor):
                for i in range(N_STEPS):
                    x = XYZ[:, i * 6 : (i + 1) * 6]
                    if i > 0:
                        nc.tensor.wait_ge(sa, 2 * (i - 1) + 1)
                    mm1 = nc.tensor.matmul(Dp[:], DIFU[:], x, start=True, stop=False)
                    if i == 0:
                        mm1._wait_ge(dma_i, 160)
                    else:
                        # data: xyz[i] ready; WAR: step i-1's ACT reads of Dp done
                        # (only one wait may ride on an instruction -> the WAR
                        # wait goes on a standalone sequencer wait just before)
                        mm1._wait_ge(sv, v_xnext(i - 1))
                    nc.tensor.matmul(
                        Dp[:, 0:3], B0[:], x[:, 3:6], start=False, stop=True
                    ).then_inc(st)
                    nc.tensor.matmul(
                        DEVp[:], DIFD[:], SVEC[:], start=True, stop=False
                    )._wait_ge(sv, v_svec(i))
                    nc.tensor.matmul(
                        DEVp[:, 3:6], B127N[:], SVEC[:, 0:3], start=False, stop=True
                    ).then_inc(st)

            @block.scalar
            def _(scalar):
                nc.scalar.wait_ge(sv, 1)  # prologue memsets (BIAS tiles) done
                for i in range(N_STEPS):
                    dl = DLEN[:, i * 2 : i * 2 + 2]
                    nc.scalar.activation(
                        SQS[:, 0:3], Dp[:, 0:3], Act.Square, accum_out=SSD[:, 0:1]
                    )._wait_ge(st, 2 * i + 1)
                    nc.scalar.activation(
                        SQS[:, 3:6], Dp[:, 3:6], Act.Square, accum_out=SSD[:, 1:2]
                    )
                    sd()
                    nc.scalar.activation(dl, SSD[:], Act.Sqrt, bias=BIAS1[:])
                    sd(sa)
                    if i == N_STEPS - 1:
                        continue
                    rnew = RES[:, (i + 1) * 6 : (i + 2) * 6]
                    nr = NRM[:, i * 2 : i * 2 + 2]
                    nc.scalar.activation(
                        SQS[:, 0:3], rnew[:, 0:3], Act.Square, accum_out=SSR[:, 0:1]
                    )._wait_ge(sv, v_rnew(i))
                    nc.scalar.activation(
                        SQS[:, 3:6], rnew[:, 3:6], Act.Square, accum_out=SSR[:, 1:2]
                    )
                    sd()
                    nc.scalar.activation(nr, SSR[:], Act.Sqrt, bias=BIAS1[:])
                    sd(sa)
                nc.scalar.wait_ge(sv, V_TSS)
                nc.scalar.activation(TLEN[:], TSS[:], Act.Sqrt, bias=BIAS0[:])
                sd(sa)

            @block.vector
            def _(vector):
                # ---------------- prologue ----------------
                v.memset(RES[:, 0:6], 0.0)
                v.memset(ONES2[:], 1.0)
                v.memset(ZERO2[:], 0.0)
                v.memset(BIAS1[:], TINY)
                v.memset(BIAS0[:], 0.0)
                v.memset(ONE96[:], 1.0)
                v.memset(ZERO96[:], 0.0)
                vd(sv)  # V = 1
                # n_safe = where(all(|n|<=1e-8), 1, n); n_mask = where(.., 0, n)
                nor32 = NOR[:].rearrange("p (g c) -> p g c", c=3)
                v.tensor_reduce(
                    ZA[:], nor32, axis=AX.X, op=Alu.max, apply_absolute_value=True
                )._wait_ge(dma_i, 160)
                vd()
                v.tensor_single_scalar(ZN[:], ZA[:], 1e-8, Alu.is_le)
                vd()
                v.tensor_copy(
                    ZNB[:].rearrange("p (g c) -> p g c", c=3),
                    ZN[:].unsqueeze(2).broadcast_to([P, 32, 3]),
                )
                v.tensor_copy(NSAFE[:], NOR[:])
                v.tensor_copy(NMASK[:], NOR[:])
                vd()
                v.copy_predicated(NSAFE[:], ZNB[:].bitcast(mybir.dt.int32), ONE96[:])
                v.copy_predicated(NMASK[:], ZNB[:].bitcast(mybir.dt.int32), ZERO96[:])
                v.tensor_single_scalar(L0Z[:], LEN0[:], 0.0, Alu.is_equal)
                vd()

                # ---------------- the 16-step chain ----------------
                for i in range(N_STEPS):
                    x = XYZ[:, i * 6 : (i + 1) * 6]
                    dl = DLEN[:, i * 2 : i * 2 + 2]
                    v.reciprocal(RINV[:], dl)._wait_ge(sa, 2 * i + 1)
                    vd()
                    rinvb = RINV[:].unsqueeze(2).broadcast_to([P, 2, 3])
                    v.tensor_tensor(
                        g3(W, 0),
                        Dp[:].rearrange("p (h c) -> p h c", h=2),
                        rinvb,
                        Alu.mult,
                    )._wait_ge(st, 2 * i + 1)
                    vd()
                    fb = FDEV[:, i * 2 : i * 2 + 2].unsqueeze(2).broadcast_to([P, 2, 3])
                    v.tensor_tensor(g3(SVEC, 0), g3(W, 0), fb, Alu.mult)
                    vd(sv)  # V_svec
                    # residual update
                    rprev = RES[:, i * 6 : (i + 1) * 6]
                    rnew = RES[:, (i + 1) * 6 : (i + 2) * 6]
                    v.tensor_tensor(R1[:], rprev, DEVp[:], Alu.subtract)._wait_ge(
                        st, 2 * i + 2
                    )
                    vd()
                    v.tensor_tensor(
                        rnew, R1[:], LOADS[:, i * 6 : (i + 1) * 6], Alu.subtract
                    )
                    vd(sv)  # V_rnew
                    if i == N_STEPS - 1:
                        break
                    r3 = g3(RES, (i + 1) * 6)
                    # zero_r mask (runs while ACT does the |res| sqrt)
                    v.tensor_reduce(
                        MABS[:], r3, axis=AX.X, op=Alu.max, apply_absolute_value=True
                    )
                    vd()
                    v.tensor_single_scalar(M[:], MABS[:], 1e-8, Alu.is_le)
                    # cos_nop needs only xyz -> interleave it here, it is
                    # independent of the sa wait below
                    v.tensor_tensor(OMX[:], ORI[:, i * 6 : (i + 1) * 6], x, Alu.subtract)
                    vd()
                    v.tensor_tensor(g3(CX, 0), g3(NMASK, i * 6), g3(OMX, 0), Alu.mult)
                    vd()
                    v.tensor_reduce(CZ[:], g3(CX, 0), axis=AX.X, op=Alu.add)
                    vd()
                    v.copy_predicated(CZ[:], M[:].bitcast(mybir.dt.int32), ZERO2[:])
                    nr = NRM[:, i * 2 : i * 2 + 2]
                    v.reciprocal(RINVR[:], nr)._wait_ge(sa, 2 * i + 2)
                    vd()
                    v.tensor_tensor(
                        g3(UNIT, 0),
                        r3,
                        RINVR[:].unsqueeze(2).broadcast_to([P, 2, 3]),
                        Alu.mult,
                    )
                    vd()
                    # denom = n_safe . unit(res_safe)  (:= 1 where zero_r)
                    v.tensor_tensor(g3(NU, 0), g3(NSAFE, i * 6), g3(UNIT, 0), Alu.mult)
                    vd()
                    v.tensor_reduce(DEN[:], g3(NU, 0), axis=AX.X, op=Alu.add)
                    vd()
                    v.copy_predicated(DEN[:], M[:].bitcast(mybir.dt.int32), ONES2[:])
                    vd()
                    v.reciprocal(RDEN[:], DEN[:])
                    vd()
                    # lengths_seq = len0 + (len0 == 0) * cos/denom
                    v.tensor_tensor(LP[:], CZ[:], RDEN[:], Alu.mult)
                    vd()
                    ln = LEN[:, i * 2 : i * 2 + 2]
                    v.tensor_tensor(LT[:], LP[:], L0Z[:, i * 2 : i * 2 + 2], Alu.mult)
                    vd()
                    v.tensor_tensor(ln, LT[:], LEN0[:, i * 2 : i * 2 + 2], Alu.add)
                    vd()
                    # xyz_next = xyz + lengths_seq * unit
                    v.tensor_tensor(
                        g3(XT, 0),
                        g3(UNIT, 0),
                        ln.unsqueeze(2).broadcast_to([P, 2, 3]),
                        Alu.mult,
                    )
                    vd()
                    v.tensor_tensor(XYZ[:, (i + 1) * 6 : (i + 2) * 6], x, XT[:], Alu.add)
                    vd(sv)  # V_xnext

                # ---------------- epilogue ----------------
                # trail lengths: |xyz[i+1] - xyz[i]|
                v.tensor_tensor(TD[:], XYZ[:, 6:96], XYZ[:, 0:90], Alu.subtract)
                vd()
                v.tensor_tensor(TSQ[:], TD[:], TD[:], Alu.mult)
                vd()
                v.tensor_reduce(
                    TSS[:],
                    TSQ[:].rearrange("p (g c) -> p g c", c=3),
                    axis=AX.X,
                    op=Alu.add,
                )
                vd(sv)  # V_TSS
                # trail forces: |res_i| * (-1 if len_i < 0 else 1)
                v.tensor_single_scalar(MNEG[:], LEN[:], 0.0, Alu.is_lt)
                vd()
                v.tensor_scalar(SGN[:], MNEG[:], -2.0, 1.0, Alu.mult, Alu.add)
                vd()
                v.tensor_tensor(TF[:], NRM[:], SGN[:], Alu.mult)
                vd(sv)  # V_TF
                # join on ACT's last write (TLEN) so out-DMAs need one wait
                v.tensor_copy(JNK[0:1, 0:1], TLEN[0:1, 0:1])._wait_ge(sa, A_TLEN)
                vd(sv)  # V_FIN


_PROGRAM_CACHE: dict = {}


def _build_program():
    if "nc" in _PROGRAM_CACHE:
        return _PROGRAM_CACHE["nc"]
    nc = bass.Bass("TRN2")
    in_aps = {
        n: nc.dram_tensor(n, list(IN_SHAPES[n]), F32, kind="ExternalInput")[:]
        for n in IN_NAMES
    }
    out_aps = {
        n: nc.dram_tensor(n, list(OUT_SHAPES[n]), F32, kind="ExternalOutput")[:]
        for n in OUT_NAMES
    }
    _emit(nc, in_aps, out_aps)
    _PROGRAM_CACHE["nc"] = nc
    return nc


# ---------------------------------------------------------------------------
# host side
# ---------------------------------------------------------------------------


def _blk(arr):  # [4096, k] node-major -> [128, 16*2*k] (p, i, h, k)
    k = arr.shape[1]
    return (
        arr.reshape(N_STEPS, H, P, k).transpose(2, 0, 1, 3).reshape(P, N_STEPS * H * k)
    ).astype(np.float32, copy=True)


def _stage_inputs(inputs):
    xyz0 = np.asarray(inputs["xyz0"], np.float32)
    loads = np.asarray(inputs["loads"], np.float32)
    lengths0 = np.asarray(inputs["lengths0"], np.float32)
    planes = np.asarray(inputs["planes"], np.float32)
    forces0 = np.asarray(inputs["forces0"], np.float32)

    difu = np.zeros((P, P), np.float32)
    for m in range(P - 1):
        difu[m + 1, m] = 1.0
    difu[np.arange(P), np.arange(P)] += -1.0
    b0 = np.zeros((P, P), np.float32)
    b0[0, 127] = 1.0
    difd = np.zeros((P, P), np.float32)
    difd[np.arange(P), np.arange(P)] = 1.0
    for m in range(1, P):
        difd[m - 1, m] = -1.0
    b127n = np.zeros((P, P), np.float32)
    b127n[127, 0] = -1.0

    fd = np.zeros((N_STEPS, N_TRAILS), np.float32)
    fd[:, : N_TRAILS - 1] = forces0[N_TRAIL_E:, 0].reshape(N_STEPS, N_TRAILS - 1)

    return {
        "xyz0_blk": np.ascontiguousarray(
            xyz0[:N_TRAILS].reshape(H, P, 3).transpose(1, 0, 2).reshape(P, 6)
        ),
        "loads_blk": _blk(loads),
        "ori_blk": _blk(planes[:, 0:3]),
        "nor_blk": _blk(planes[:, 3:6]),
        "len0_blk": _blk(lengths0),
        "fdev_blk": np.ascontiguousarray(
            fd.reshape(N_STEPS, H, P).transpose(2, 0, 1).reshape(P, 32)
        ),
        "difu_m": difu,
        "b0_m": b0,
        "difd_m": difd,
        "b127n_m": b127n,
    }


def _assemble_outputs(dev, inputs):
    loads = np.asarray(inputs["loads"], np.float32)
    forces0 = np.asarray(inputs["forces0"], np.float32)

    xyz = (
        dev["out_xyz"]
        .reshape(P, N_STEPS, H, 3)
        .transpose(1, 2, 0, 3)
        .reshape(N_NODES, 3)
        .copy()
    )
    reactions = np.zeros((N_NODES, 3), np.float32)
    reactions[N_NODES - N_TRAILS :] = (
        dev["out_res"].reshape(P, H, 3).transpose(1, 0, 2).reshape(N_TRAILS, 3)
    )
    lengths = np.empty((N_EDGES, 1), np.float32)
    lengths[:N_TRAIL_E, 0] = (
        dev["out_tlen"].reshape(P, N_STEPS - 1, H).transpose(1, 2, 0).reshape(-1)
    )
    lengths[N_TRAIL_E:, 0] = (
        dev["out_dlen"]
        .reshape(P, N_STEPS, H)
        .transpose(1, 2, 0)
        .reshape(N_STEPS, N_TRAILS)[:, : N_TRAILS - 1]
        .reshape(-1)
    )
    forces = np.where(forces0 != 0.0, forces0, 0.0).astype(np.float32)
    forces[:N_TRAIL_E, 0] = (
        dev["out_tf"].reshape(P, N_STEPS - 1, H).transpose(1, 2, 0).reshape(-1)
    )
    return xyz, reactions, lengths, loads.copy(), forces


def _structure_ok(inputs):
    try:
        seq = np.asarray(inputs["sequences"])
        seqe = np.asarray(inputs["sequences_edges"])
        dev = np.asarray(inputs["deviation_edges"])
        C = np.asarray(inputs["connectivity"])
        inc = np.asarray(inputs["incidence"])
        if not np.array_equal(
            seq, np.arange(N_NODES, dtype=seq.dtype).reshape(N_STEPS, N_TRAILS)
        ):
            return False
        if not np.array_equal(
            seqe, np.arange(N_TRAIL_E, dtype=seqe.dtype).reshape(N_STEPS - 1, N_TRAILS)
        ):
            return False
        expected_dev = np.zeros(N_EDGES, np.float32)
        expected_dev[N_TRAIL_E:] = 1.0
        if not np.array_equal(dev, expected_dev):
            return False
        # endpoints of every edge in builder order
        it = np.arange(N_TRAIL_E)
        u_t = (it // N_TRAILS) * N_TRAILS + (it % N_TRAILS)
        v_t = u_t + N_TRAILS
        id_ = np.arange(N_DEV_E)
        u_d = (id_ // (N_TRAILS - 1)) * N_TRAILS + (id_ % (N_TRAILS - 1))
        v_d = u_d + 1
        u = np.concatenate([u_t, u_d])
        v = np.concatenate([v_t, v_d])
        e = np.arange(N_EDGES)
        if C.shape != (N_EDGES, N_NODES) or inc.shape != (N_EDGES, N_NODES):
            return False
        if not (
            np.all(C[e, u] == -1.0)
            and np.all(C[e, v] == 1.0)
            and np.count_nonzero(C) == 2 * N_EDGES
        ):
            return False
        if not (
            np.all(inc[e, u] == 1.0)
            and np.all(inc[e, v] == -1.0)
            and np.count_nonzero(inc) == 2 * N_EDGES
        ):
            return False
        return True
    except Exception:
        return False


def _numpy_reference(
    xyz0,
    loads,
    lengths0,
    planes,
    forces0,
    connectivity,
    incidence,
    deviation_edges,
    sequences,
    sequences_edges,
):
    """Pure numpy port of the jax reference — emergency fallback only."""
    xyz0 = np.asarray(xyz0, np.float32)
    loads = np.asarray(loads, np.float32)
    lengths0 = np.asarray(lengths0, np.float32)
    planes = np.asarray(planes, np.float32)
    forces0 = np.asarray(forces0, np.float32)
    C = np.asarray(connectivity, np.float32)
    inc = np.asarray(incidence, np.float32)
    devm = np.asarray(deviation_edges, np.float32)
    seq = np.asarray(sequences)
    seqe = np.asarray(sequences_edges)

    def safe_normalize(vv):
        ss = np.sum(vv * vv, axis=-1, keepdims=True)
        return vv / np.sqrt(np.where(ss == 0.0, 1.0, ss))

    n_nodes = loads.shape[0]
    n_trails = seq.shape[1]
    dev = devm[:, None]
    xyz = np.zeros((n_nodes + 1, 3), np.float32)
    xyz_seq_in = xyz0[seq[0], :].copy()
    residuals = np.zeros((n_trails, 3), np.float32)
    res_list, len_list = [], []
    for sequence in seq:
        xyz[sequence, :] = xyz_seq_in
        xyz_seq = xyz[sequence, :]
        padded = (sequence < 0)[:, None]
        vectors = safe_normalize(C @ xyz[:-1])
        inc_seq = inc[:, sequence] * dev
        f = forces0 * inc_seq
        deviation = np.einsum("es,ed->sd", f, vectors).astype(np.float32)
        res_new = residuals - deviation - loads[sequence, :]
        residuals = np.where(padded, residuals, res_new).astype(np.float32)
        plane = planes[sequence, :]
        origin, normal = plane[:, :3], plane[:, 3:]
        zero_n = np.all(np.isclose(normal, 0.0), axis=-1, keepdims=True)
        normal = np.where(zero_n, 1.0, normal).astype(np.float32)
        cos_nop = np.where(
            zero_n[:, 0], 0.0, np.sum(normal * (origin - xyz_seq), axis=-1)
        ).astype(np.float32)
        zero_r = np.all(np.isclose(residuals, 0.0), axis=-1, keepdims=True)
        res_safe = np.where(zero_r, 1.0, residuals).astype(np.float32)
        denom = np.sum(normal * safe_normalize(res_safe), axis=-1)
        len_plane = np.where(zero_r[:, 0], 0.0, cos_nop / denom).astype(np.float32)
        len_signed = lengths0[sequence].ravel()
        lengths_seq = np.where(len_signed != 0.0, len_signed, len_plane).astype(
            np.float32
        )
        xyz_next = xyz_seq + lengths_seq[:, None] * safe_normalize(residuals)
        xyz_next = np.where(padded, xyz_seq, xyz_next).astype(np.float32)
        res_list.append(residuals.copy())
        len_list.append(lengths_seq.copy())
        xyz_seq_in = xyz_next
    res_all = np.stack(res_list)
    len_all = np.stack(len_list)
    xyz = xyz[:-1]
    reactions = np.zeros((n_nodes, 3), np.float32)
    reactions[seq[-1], :] = res_all[-1]
    forces = np.where(forces0 != 0.0, forces0, 0.0).astype(np.float32)
    res_trail = res_all[:-1].reshape(-1, 3)
    len_trail = len_all[:-1].reshape(-1, 1)
    trail_forces = np.linalg.norm(res_trail, axis=-1, keepdims=True).astype(
        np.float32
    ) * np.where(len_trail < 0.0, -1.0, 1.0).astype(np.float32)
    forces[seqe.ravel(), :] = trail_forces
    lengths = np.linalg.norm(C @ xyz, axis=-1, keepdims=True).astype(np.float32)
    return xyz, reactions, lengths, loads, forces


N_CORES = 8

# Filled after every device run: the BassKernelResults (exec_time_ns is
# populated when tracing is enabled, e.g. BASS_TRACE=1).
LAST_RUN = None


def kernel(**inputs):
    global LAST_RUN
    if not _structure_ok(inputs):
        return _numpy_reference(**inputs)
    staged = _stage_inputs(inputs)
    nc = _build_program()
    in_maps = [dict(staged) for _ in range(N_CORES)]
    res = run_bass_kernel_spmd(nc, in_maps, core_ids=list(range(N_CORES)))
    LAST_RUN = res
    dev = res.results[0]
    return _assemble_outputs(dev, inputs)


if __name__ == "__main__":
    sys.path.insert(0, "/root/problem")
    import reference

    inputs = {k: np.asarray(v) for k, v in reference.setup_inputs().items()}
    outs = kernel(**inputs)
    print([o.shape for o in outs])


# revision 13
# speedup vs baseline: 1.6620x; 1.6620x over previous
"""Trainium2 Bass kernel for nn_EquilibriumModel (gnn_message_passing).

Strategy
--------
The model is a 16-step sequential equilibrium scan over a 16x256 "trail grid"
graph.  The heavy inputs (connectivity / incidence, each [7920, 4096]) are the
signed incidence matrices of a fixed, deterministic topology: trail edges
(i,t)->(i+1,t) and deviation edges (i,t)->(i,t+1), with `sequences` equal to
arange.  kernel() verifies that topology on the host (falling back to a pure
numpy port of the reference if anything differs) and then runs a specialized
on-chip program:

  - partition dim = t (mod 128), free dim = (step, half, xyz-comp)
  - the per-step deviation-force stencil (t -> t+-1 shifts) is done on the
    TensorEngine with constant +-1 shift matrices accumulated in PSUM
    (compute engines cannot address partition offsets != 0/32/64/96)
  - sum-of-squares + sqrt via ScalarEngine Square(accum_out)/Sqrt, the rest of
    the per-step chain (normalize, plane intersection, position update) on the
    VectorEngine
  - edge lengths / trail forces are produced in bulk from the accumulated
    position history; host code only re-lays-out device blocks and passes
    through the outputs the reference returns verbatim (loads, dev-edge
    forces, zero rows of reactions).

The same program is replicated SPMD on all 8 NeuronCores (the sequential
scan does not shard without paying a ~10us collective floor per step, which
would dominate); core 0's outputs are used.
"""

import sys

import numpy as np

for _p in ("/opt/trn_rl_repo",):
    if _p not in sys.path:
        sys.path.insert(0, _p)

import concourse.bass as bass  # noqa: E402
import concourse.tile as tile  # noqa: E402
from concourse import mybir  # noqa: E402
from concourse.bass_utils import run_bass_kernel_spmd  # noqa: E402

F32 = mybir.dt.float32
Alu = mybir.AluOpType
Act = mybir.ActivationFunctionType
AX = mybir.AxisListType

N_TRAILS = 256
N_STEPS = 16
N_NODES = N_TRAILS * N_STEPS  # 4096
N_TRAIL_E = (N_STEPS - 1) * N_TRAILS  # 3840
N_DEV_E = N_STEPS * (N_TRAILS - 1)  # 4080
N_EDGES = N_TRAIL_E + N_DEV_E  # 7920
P = 128
H = 2  # halves of the 256 trails on the 128 partitions: t = h*128 + p

IN_NAMES = [
    "xyz0_blk",  # [128, 6]    step-0 positions, (p, h, c), t = 2p+h
    "loads_blk",  # [128, 96]  (p, i, h, c)
    "ori_blk",  # [128, 96]
    "nor_blk",  # [128, 96]
    "len0_blk",  # [128, 32]   (p, i, h)
    "fdev_blk",  # [128, 32]   deviation-edge forces, t=255 slot zeroed
    "sup_m",  # [128, 128]   out[m] = in[m+1]  (pure shift up)
    "sdn_m",  # [128, 128]   out[m] = in[m-1]  (pure shift down)
]
OUT_NAMES = [
    "out_xyz",  # [128, 96]   (p, i, h, c)
    "out_res",  # [128, 6]    residuals after step 15, (p, h, c)
    "out_dlen",  # [128, 32]  deviation-edge lengths (p, i, h)
    "out_tlen",  # [128, 30]  trail-edge lengths (p, i<15, h)
    "out_tf",  # [128, 30]    trail forces (p, i<15, h)
]
IN_SHAPES = {
    "xyz0_blk": (P, 6),
    "loads_blk": (P, 96),
    "ori_blk": (P, 96),
    "nor_blk": (P, 96),
    "len0_blk": (P, 32),
    "fdev_blk": (P, 32),
    "sup_m": (P, P),
    "sdn_m": (P, P),
}
OUT_SHAPES = {
    "out_xyz": (P, 96),
    "out_res": (P, 6),
    "out_dlen": (P, 32),
    "out_tlen": (P, 30),
    "out_tf": (P, 30),
}

TINY = 1e-38  # < ulp of any ss we care about; keeps ln() finite at ss = 0

# Emit explicit per-op DRAIN instructions on DVE/ACT.  The engines execute
# their streams in order and interlock same-engine hazards in hardware
# (Tile-generated kernels carry almost no drains), so this stays off; the
# flag exists to flip on if a race is ever suspected.
EMIT_DRAINS = False


def _emit(nc, ins: dict, outs: dict):
    """Emit the whole program in raw bass (no Tile scheduler).

    This walrus build rejects any DVE-family (S3S3D3) instruction carrying
    more than ONE semaphore wait, which Tile's scheduler freely emits, so
    semaphores are placed by hand:

      - sem dma_i: every input DMA incs it by 16 (single wait, value 128,
        observes ALL input loads at once)
      - sems st/sa/sv: progress counters on PE / ACT / DVE, bumped at the
        few spots another engine waits on; per-engine program order covers
        everything else.  Every instruction carries <= 1 wait.

    Layout: partition p, halves h; trail t = 2p + h; free dim (step i, h, c).
    With this interleaving the t+-1 stencil needs only ONE PE shift matmul
    per direction (neighbor of even t is on the same partition), applied to
    the odd/even half [128, 3].

    rsqrt/sqrt: the ACT Sqrt LUT is only ~6e-3 accurate on TRN2, but Ln/Exp
    are ~1e-5; rsqrt(ss) = Exp(-0.5 * Ln(ss + 1e-38)) and norms are then
    ss * rsqrt(ss) on DVE (exact 0 at ss = 0).  DVE reciprocal (~6e-8) does
    the one true division.

    Per-step semaphore targets (1-based):
      PE:  T_up(i) = 2i+1, T_dn(i) = 2i+2
      ACT: A_r1(i) = 2i+1, A_r1r(i) = 2i+2 (i<15); epilogue A_TLEN = 32
      DVE: V = 1 after prologue memsets; step i < 15:
             V_ss = 5i+2, V_svec = 5i+3, V_rnew = 5i+4, V_ssr = 5i+5,
             V_xnext = 5i+6
           step 15: V_ss = 77, V_svec = 78, V_rnew = 79
           epilogue: V_TSS = 80, V_TF = 81, V_FIN = 82
    """
    from contextlib import ExitStack

    with ExitStack() as ctx:

        def t(tag, shape):
            return ctx.enter_context(nc.sbuf_tensor(tag, list(shape), F32))

        # persistent state/history
        XYZ = t("XYZ", (P, 96))
        RES = t("RES", (P, 102))  # 17 slots of (h,c); slot 0 = zeros
        LEN = t("LEN", (P, 30))
        NRM = t("NRM", (P, 30))
        DLEN = t("DLEN", (P, 32))
        # inputs
        LOADS = t("LOADS", (P, 96))
        ORI = t("ORI", (P, 96))
        NOR = t("NOR", (P, 96))
        LEN0 = t("LEN0", (P, 32))
        FDEV = t("FDEV", (P, 32))
        SUP = t("SUP", (P, P))
        SDN = t("SDN", (P, P))
        # derived statics
        NSAFE = t("NSAFE", (P, 96))
        NMASK = t("NMASK", (P, 96))
        L0Z = t("L0Z", (P, 32))
        ZA = t("ZA", (P, 32))
        ZN = t("ZN", (P, 32))
        ZNB = t("ZNB", (P, 96))
        ONE96 = t("ONE96", (P, 96))
        ZERO96 = t("ZERO96", (P, 96))
        ONES2 = t("ONES2", (P, 2))
        ZERO2 = t("ZERO2", (P, 2))
        BIAS1 = t("BIAS1", (P, 1))
        # per-step scratch
        D = t("D", (P, 6))
        SQS = t("SQS", (P, 6))
        SS = t("SS", (P, 2))
        LNS = t("LNS", (P, 2))
        R1D = t("R1D", (P, 2))
        W = t("W", (P, 6))
        SVEC = t("SVEC", (P, 6))
        DEV = t("DEV", (P, 6))
        RR1 = t("RR1", (P, 6))
        MABS = t("MABS", (P, 2))
        M = t("M", (P, 2))
        SQR = t("SQR", (P, 6))
        SSR = t("SSR", (P, 2))
        LNR = t("LNR", (P, 2))
        R1R = t("R1R", (P, 2))
        UNIT = t("UNIT", (P, 6))
        NU = t("NU", (P, 6))
        DEN = t("DEN", (P, 2))
        RDEN = t("RDEN", (P, 2))
        OMX = t("OMX", (P, 6))
        CX = t("CX", (P, 6))
        CZ = t("CZ", (P, 2))
        LP = t("LP", (P, 2))
        LT = t("LT", (P, 2))
        XT = t("XT", (P, 6))
        # epilogue scratch
        TD = t("TD", (P, 90))
        TSQ = t("TSQ", (P, 90))
        TSS = t("TSS", (P, 30))
        LNT = t("LNT", (P, 30))
        R1T = t("R1T", (P, 30))
        TLEN = t("TLEN", (P, 30))
        MNEG = t("MNEG", (P, 30))
        SGN = t("SGN", (P, 30))
        TF = t("TF", (P, 30))

        SHp = ctx.enter_context(nc.psum_tensor("SHp", [P, 3], F32))
        SHDp = ctx.enter_context(nc.psum_tensor("SHDp", [P, 3], F32))

        dma_i = nc.alloc_semaphore("dma_in")
        dma_o = nc.alloc_semaphore("dma_out")
        sv = nc.alloc_semaphore("sv")  # DVE progress
        sa = nc.alloc_semaphore("sa")  # ACT progress
        st = nc.alloc_semaphore("st")  # PE progress

        def g3(T, off6):  # [128, 2, 3] view at (h,c) group offset
            return T[:, off6 : off6 + 6].rearrange("p (h c) -> p h c", h=2)

        def v_ss(i):
            return 5 * i + 2 if i < 15 else 77

        def v_svec(i):
            return 5 * i + 3 if i < 15 else 78

        def v_rnew(i):
            return 5 * i + 4 if i < 15 else 79

        def v_ssr(i):
            return 5 * i + 5

        def v_xnext(i):
            return 5 * i + 6

        V_TSS, V_TF, V_FIN = 80, 81, 82
        A_TLEN = 32

        v = nc.vector
        s = nc.scalar

        def vd(inc=None):
            if EMIT_DRAINS:
                d = v.drain()
                if inc is not None:
                    d.then_inc(inc)
            elif inc is not None:
                v.drain().then_inc(inc)

        def sd(inc=None):
            if EMIT_DRAINS:
                d = s.drain()
                if inc is not None:
                    d.then_inc(inc)
            elif inc is not None:
                s.drain().then_inc(inc)

        with nc.Block() as block:

            @block.sync
            def _(sync):
                for name, dst in [
                    ("xyz0_blk", XYZ[:, 0:6]),
                    ("loads_blk", LOADS[:]),
                    ("ori_blk", ORI[:]),
                    ("nor_blk", NOR[:]),
                    ("len0_blk", LEN0[:]),
                    ("fdev_blk", FDEV[:]),
                    ("sup_m", SUP[:]),
                    ("sdn_m", SDN[:]),
                ]:
                    nc.sync.dma_start(dst, ins[name][:]).then_inc(dma_i, 16)
                nc.sync.wait_ge(sv, V_FIN)
                nc.sync.dma_start(outs["out_xyz"][:], XYZ[:]).then_inc(dma_o, 16)
                nc.sync.dma_start(outs["out_res"][:], RES[:, 96:102]).then_inc(dma_o, 16)
                nc.sync.dma_start(outs["out_dlen"][:], DLEN[:]).then_inc(dma_o, 16)
                nc.sync.dma_start(outs["out_tlen"][:], TLEN[:]).then_inc(dma_o, 16)
                nc.sync.dma_start(outs["out_tf"][:], TF[:]).then_inc(dma_o, 16)
                nc.sync.wait_ge(dma_o, 80)

            @block.tensor
            def _(tensor):
                for i in range(N_STEPS):
                    xh0 = XYZ[:, i * 6 : i * 6 + 3]
                    mm = nc.tensor.matmul(SHp[:], SUP[:], xh0, start=True, stop=True)
                    if i == 0:
                        mm._wait_ge(dma_i, 128)
                    else:
                        mm._wait_ge(sv, v_xnext(i - 1))
                    nc.tensor.drain().then_inc(st)
                    nc.tensor.matmul(
                        SHDp[:], SDN[:], SVEC[:, 3:6], start=True, stop=True
                    )._wait_ge(sv, v_svec(i))
                    nc.tensor.drain().then_inc(st)

            @block.scalar
            def _(scalar):
                for i in range(N_STEPS):
                    nc.scalar.activation(LNS[:], SS[:], Act.Ln, bias=BIAS1[:])._wait_ge(
                        sv, v_ss(i)
                    )
                    sd()
                    nc.scalar.activation(R1D[:], LNS[:], Act.Exp, scale=-0.5)
                    sd(sa)
                    if i == N_STEPS - 1:
                        continue
                    nc.scalar.activation(LNR[:], SSR[:], Act.Ln, bias=BIAS1[:])._wait_ge(
                        sv, v_ssr(i)
                    )
                    sd()
                    nc.scalar.activation(R1R[:], LNR[:], Act.Exp, scale=-0.5)
                    sd(sa)
                nc.scalar.activation(LNT[:], TSS[:], Act.Ln, bias=BIAS1[:])._wait_ge(
                    sv, V_TSS
                )
                sd()
                nc.scalar.activation(R1T[:], LNT[:], Act.Exp, scale=-0.5)
                sd(sa)  # A_TLEN

            @block.vector
            def _(vector):
                # ---------------- prologue ----------------
                v.memset(RES[:, 0:6], 0.0)
                v.memset(ONES2[:], 1.0)
                v.memset(ZERO2[:], 0.0)
                v.memset(BIAS1[:], TINY)
                v.memset(ONE96[:], 1.0)
                v.memset(ZERO96[:], 0.0)
                vd(sv)  # V = 1
                nor32 = NOR[:].rearrange("p (g c) -> p g c", c=3)
                v.tensor_reduce(
                    ZA[:], nor32, axis=AX.X, op=Alu.max, apply_absolute_value=True
                )._wait_ge(dma_i, 128)
                vd()
                v.tensor_single_scalar(ZN[:], ZA[:], 1e-8, Alu.is_le)
                vd()
                v.tensor_copy(
                    ZNB[:].rearrange("p (g c) -> p g c", c=3),
                    ZN[:].unsqueeze(2).broadcast_to([P, 32, 3]),
                )
                v.tensor_copy(NSAFE[:], NOR[:])
                v.tensor_copy(NMASK[:], NOR[:])
                vd()
                v.copy_predicated(NSAFE[:], ZNB[:].bitcast(mybir.dt.int32), ONE96[:])
                v.copy_predicated(NMASK[:], ZNB[:].bitcast(mybir.dt.int32), ZERO96[:])
                v.tensor_single_scalar(L0Z[:], LEN0[:], 0.0, Alu.is_equal)
                vd()

                # ---------------- the 16-step chain ----------------
                for i in range(N_STEPS):
                    x = XYZ[:, i * 6 : (i + 1) * 6]
                    xh0 = XYZ[:, i * 6 : i * 6 + 3]
                    xh1 = XYZ[:, i * 6 + 3 : i * 6 + 6]
                    # deviation-edge vectors: d(2p)   = x_h1[p] - x_h0[p]
                    #                         d(2p+1) = x_h0[p+1] - x_h1[p]
                    v.tensor_tensor(D[:, 0:3], xh1, xh0, Alu.subtract)
                    vd()
                    v.tensor_tensor(D[:, 3:6], SHp[:], xh1, Alu.subtract)._wait_ge(
                        st, 2 * i + 1
                    )
                    vd()
                    v.tensor_tensor(SQS[:], D[:], D[:], Alu.mult)
                    vd()
                    v.tensor_reduce(
                        SS[:], g3(SQS, 0), axis=AX.X, op=Alu.add
                    )
                    vd(sv)  # V_ss
                    # cos_nop pieces only need x -> overlap the ACT ln/exp
                    v.tensor_tensor(OMX[:], ORI[:, i * 6 : (i + 1) * 6], x, Alu.subtract)
                    vd()
                    v.tensor_tensor(g3(CX, 0), g3(NMASK, i * 6), g3(OMX, 0), Alu.mult)
                    vd()
                    v.tensor_reduce(CZ[:], g3(CX, 0), axis=AX.X, op=Alu.add)
                    vd()
                    v.tensor_tensor(
                        g3(W, 0),
                        g3(D, 0),
                        R1D[:].unsqueeze(2).broadcast_to([P, 2, 3]),
                        Alu.mult,
                    )._wait_ge(sa, 2 * i + 1)
                    vd()
                    fb = FDEV[:, i * 2 : i * 2 + 2].unsqueeze(2).broadcast_to([P, 2, 3])
                    v.tensor_tensor(g3(SVEC, 0), g3(W, 0), fb, Alu.mult)
                    vd(sv)  # V_svec
                    # dev-edge length output (= ss * rsqrt(ss)), off the path
                    v.tensor_tensor(DLEN[:, i * 2 : i * 2 + 2], SS[:], R1D[:], Alu.mult)
                    vd()
                    # deviation: dev(2p) = s(2p) - s(2p-1) = svec_h0[p] - svec_h1[p-1]
                    #            dev(2p+1) = s(2p+1) - s(2p) = svec_h1[p] - svec_h0[p]
                    v.tensor_tensor(
                        DEV[:, 3:6], SVEC[:, 3:6], SVEC[:, 0:3], Alu.subtract
                    )
                    vd()
                    v.tensor_tensor(DEV[:, 0:3], SVEC[:, 0:3], SHDp[:], Alu.subtract)._wait_ge(
                        st, 2 * i + 2
                    )
                    vd()
                    rprev = RES[:, i * 6 : (i + 1) * 6]
                    rnew = RES[:, (i + 1) * 6 : (i + 2) * 6]
                    v.tensor_tensor(RR1[:], rprev, DEV[:], Alu.subtract)
                    vd()
                    v.tensor_tensor(
                        rnew, RR1[:], LOADS[:, i * 6 : (i + 1) * 6], Alu.subtract
                    )
                    vd(sv)  # V_rnew
                    if i == N_STEPS - 1:
                        break
                    r3 = g3(RES, (i + 1) * 6)
                    v.tensor_tensor(SQR[:], rnew, rnew, Alu.mult)
                    vd()
                    v.tensor_reduce(SSR[:], g3(SQR, 0), axis=AX.X, op=Alu.add)
                    vd(sv)  # V_ssr
                    # overlap ACT lnr/expr with the zero_r mask work
                    v.tensor_reduce(
                        MABS[:], r3, axis=AX.X, op=Alu.max, apply_absolute_value=True
                    )
                    vd()
                    v.tensor_single_scalar(M[:], MABS[:], 1e-8, Alu.is_le)
                    vd()
                    v.copy_predicated(CZ[:], M[:].bitcast(mybir.dt.int32), ZERO2[:])
                    vd()
                    v.tensor_tensor(
                        g3(UNIT, 0),
                        r3,
                        R1R[:].unsqueeze(2).broadcast_to([P, 2, 3]),
                        Alu.mult,
                    )._wait_ge(sa, 2 * i + 2)
                    vd()
                    v.tensor_tensor(NRM[:, i * 2 : i * 2 + 2], SSR[:], R1R[:], Alu.mult)
                    vd()
                    v.tensor_tensor(g3(NU, 0), g3(NSAFE, i * 6), g3(UNIT, 0), Alu.mult)
                    vd()
                    v.tensor_reduce(DEN[:], g3(NU, 0), axis=AX.X, op=Alu.add)
                    vd()
                    v.copy_predicated(DEN[:], M[:].bitcast(mybir.dt.int32), ONES2[:])
                    vd()
                    v.reciprocal(RDEN[:], DEN[:])
                    vd()
                    v.tensor_tensor(LP[:], CZ[:], RDEN[:], Alu.mult)
                    vd()
                    ln = LEN[:, i * 2 : i * 2 + 2]
                    v.tensor_tensor(LT[:], LP[:], L0Z[:, i * 2 : i * 2 + 2], Alu.mult)
                    vd()
                    v.tensor_tensor(ln, LT[:], LEN0[:, i * 2 : i * 2 + 2], Alu.add)
                    vd()
                    v.tensor_tensor(
                        g3(XT, 0),
                        g3(UNIT, 0),
                        ln.unsqueeze(2).broadcast_to([P, 2, 3]),
                        Alu.mult,
                    )
                    vd()
                    v.tensor_tensor(XYZ[:, (i + 1) * 6 : (i + 2) * 6], x, XT[:], Alu.add)
                    vd(sv)  # V_xnext

                # ---------------- epilogue ----------------
                v.tensor_tensor(TD[:], XYZ[:, 6:96], XYZ[:, 0:90], Alu.subtract)
                vd()
                v.tensor_tensor(TSQ[:], TD[:], TD[:], Alu.mult)
                vd()
                v.tensor_reduce(
                    TSS[:],
                    TSQ[:].rearrange("p (g c) -> p g c", c=3),
                    axis=AX.X,
                    op=Alu.add,
                )
                vd(sv)  # V_TSS
                v.tensor_single_scalar(MNEG[:], LEN[:], 0.0, Alu.is_lt)
                vd()
                v.tensor_scalar(SGN[:], MNEG[:], -2.0, 1.0, Alu.mult, Alu.add)
                vd()
                v.tensor_tensor(TF[:], NRM[:], SGN[:], Alu.mult)
                vd(sv)  # V_TF
                v.tensor_tensor(TLEN[:], TSS[:], R1T[:], Alu.mult)._wait_ge(sa, A_TLEN)
                vd(sv)  # V_FIN


_PROGRAM_CACHE: dict = {}


def _build_program():
    if "nc" in _PROGRAM_CACHE:
        return _PROGRAM_CACHE["nc"]
    nc = bass.Bass("TRN2")
    in_aps = {
        n: nc.dram_tensor(n, list(IN_SHAPES[n]), F32, kind="ExternalInput")[:]
        for n in IN_NAMES
    }
    out_aps = {
        n: nc.dram_tensor(n, list(OUT_SHAPES[n]), F32, kind="ExternalOutput")[:]
        for n in OUT_NAMES
    }
    _emit(nc, in_aps, out_aps)
    _PROGRAM_CACHE["nc"] = nc
    return nc


# ---------------------------------------------------------------------------
# host side
# ---------------------------------------------------------------------------


def _blk(arr):  # [4096, k] node-major -> [128, 16*2*k]; node = i*256 + 2p + h
    k = arr.shape[1]
    return np.ascontiguousarray(
        arr.reshape(N_STEPS, P, H, k).transpose(1, 0, 2, 3)
    ).reshape(P, N_STEPS * H * k).astype(np.float32)


def _stage_inputs(inputs):
    xyz0 = np.asarray(inputs["xyz0"], np.float32)
    loads = np.asarray(inputs["loads"], np.float32)
    lengths0 = np.asarray(inputs["lengths0"], np.float32)
    planes = np.asarray(inputs["planes"], np.float32)
    forces0 = np.asarray(inputs["forces0"], np.float32)

    sup = np.zeros((P, P), np.float32)  # out[m] = in[m+1]
    sup[np.arange(1, P), np.arange(P - 1)] = 1.0
    sdn = np.zeros((P, P), np.float32)  # out[m] = in[m-1]
    sdn[np.arange(P - 1), np.arange(1, P)] = 1.0

    fd = np.zeros((N_STEPS, N_TRAILS), np.float32)
    fd[:, : N_TRAILS - 1] = forces0[N_TRAIL_E:, 0].reshape(N_STEPS, N_TRAILS - 1)

    return {
        "xyz0_blk": np.ascontiguousarray(xyz0[:N_TRAILS].reshape(P, 6)),
        "loads_blk": _blk(loads),
        "ori_blk": _blk(planes[:, 0:3]),
        "nor_blk": _blk(planes[:, 3:6]),
        "len0_blk": _blk(lengths0),
        "fdev_blk": np.ascontiguousarray(
            fd.reshape(N_STEPS, P, H).transpose(1, 0, 2)
        ).reshape(P, 32),
        "sup_m": sup,
        "sdn_m": sdn,
    }


def _assemble_outputs(dev, inputs):
    loads = np.asarray(inputs["loads"], np.float32)
    forces0 = np.asarray(inputs["forces0"], np.float32)

    xyz = (
        dev["out_xyz"]
        .reshape(P, N_STEPS, H, 3)
        .transpose(1, 0, 2, 3)
        .reshape(N_NODES, 3)
        .copy()
    )
    reactions = np.zeros((N_NODES, 3), np.float32)
    reactions[N_NODES - N_TRAILS :] = dev["out_res"].reshape(N_TRAILS, 3)
    lengths = np.empty((N_EDGES, 1), np.float32)
    lengths[:N_TRAIL_E, 0] = (
        dev["out_tlen"].reshape(P, N_STEPS - 1, H).transpose(1, 0, 2).reshape(-1)
    )
    lengths[N_TRAIL_E:, 0] = (
        dev["out_dlen"]
        .reshape(P, N_STEPS, H)
        .transpose(1, 0, 2)
        .reshape(N_STEPS, N_TRAILS)[:, : N_TRAILS - 1]
        .reshape(-1)
    )
    forces = np.where(forces0 != 0.0, forces0, 0.0).astype(np.float32)
    forces[:N_TRAIL_E, 0] = (
        dev["out_tf"].reshape(P, N_STEPS - 1, H).transpose(1, 0, 2).reshape(-1)
    )
    return xyz, reactions, lengths, loads.copy(), forces


def _structure_ok(inputs):
    try:
        seq = np.asarray(inputs["sequences"])
        seqe = np.asarray(inputs["sequences_edges"])
        dev = np.asarray(inputs["deviation_edges"])
        C = np.asarray(inputs["connectivity"])
        inc = np.asarray(inputs["incidence"])
        if not np.array_equal(
            seq, np.arange(N_NODES, dtype=seq.dtype).reshape(N_STEPS, N_TRAILS)
        ):
            return False
        if not np.array_equal(
            seqe, np.arange(N_TRAIL_E, dtype=seqe.dtype).reshape(N_STEPS - 1, N_TRAILS)
        ):
            return False
        expected_dev = np.zeros(N_EDGES, np.float32)
        expected_dev[N_TRAIL_E:] = 1.0
        if not np.array_equal(dev, expected_dev):
            return False
        # endpoints of every edge in builder order
        it = np.arange(N_TRAIL_E)
        u_t = (it // N_TRAILS) * N_TRAILS + (it % N_TRAILS)
        v_t = u_t + N_TRAILS
        id_ = np.arange(N_DEV_E)
        u_d = (id_ // (N_TRAILS - 1)) * N_TRAILS + (id_ % (N_TRAILS - 1))
        v_d = u_d + 1
        u = np.concatenate([u_t, u_d])
        v = np.concatenate([v_t, v_d])
        e = np.arange(N_EDGES)
        if C.shape != (N_EDGES, N_NODES) or inc.shape != (N_EDGES, N_NODES):
            return False
        if not (
            np.all(C[e, u] == -1.0)
            and np.all(C[e, v] == 1.0)
            and np.count_nonzero(C) == 2 * N_EDGES
        ):
            return False
        if not (
            np.all(inc[e, u] == 1.0)
            and np.all(inc[e, v] == -1.0)
            and np.count_nonzero(inc) == 2 * N_EDGES
        ):
            return False
        return True
    except Exception:
        return False


def _numpy_reference(
    xyz0,
    loads,
    lengths0,
    planes,
    forces0,
    connectivity,
    incidence,
    deviation_edges,
    sequences,
    sequences_edges,
):
    """Pure numpy port of the jax reference — emergency fallback only."""
    xyz0 = np.asarray(xyz0, np.float32)
    loads = np.asarray(loads, np.float32)
    lengths0 = np.asarray(lengths0, np.float32)
    planes = np.asarray(planes, np.float32)
    forces0 = np.asarray(forces0, np.float32)
    C = np.asarray(connectivity, np.float32)
    inc = np.asarray(incidence, np.float32)
    devm = np.asarray(deviation_edges, np.float32)
    seq = np.asarray(sequences)
    seqe = np.asarray(sequences_edges)

    def safe_normalize(vv):
        ss = np.sum(vv * vv, axis=-1, keepdims=True)
        return vv / np.sqrt(np.where(ss == 0.0, 1.0, ss))

    n_nodes = loads.shape[0]
    n_trails = seq.shape[1]
    dev = devm[:, None]
    xyz = np.zeros((n_nodes + 1, 3), np.float32)
    xyz_seq_in = xyz0[seq[0], :].copy()
    residuals = np.zeros((n_trails, 3), np.float32)
    res_list, len_list = [], []
    for sequence in seq:
        xyz[sequence, :] = xyz_seq_in
        xyz_seq = xyz[sequence, :]
        padded = (sequence < 0)[:, None]
        vectors = safe_normalize(C @ xyz[:-1])
        inc_seq = inc[:, sequence] * dev
        f = forces0 * inc_seq
        deviation = np.einsum("es,ed->sd", f, vectors).astype(np.float32)
        res_new = residuals - deviation - loads[sequence, :]
        residuals = np.where(padded, residuals, res_new).astype(np.float32)
        plane = planes[sequence, :]
        origin, normal = plane[:, :3], plane[:, 3:]
        zero_n = np.all(np.isclose(normal, 0.0), axis=-1, keepdims=True)
        normal = np.where(zero_n, 1.0, normal).astype(np.float32)
        cos_nop = np.where(
            zero_n[:, 0], 0.0, np.sum(normal * (origin - xyz_seq), axis=-1)
        ).astype(np.float32)
        zero_r = np.all(np.isclose(residuals, 0.0), axis=-1, keepdims=True)
        res_safe = np.where(zero_r, 1.0, residuals).astype(np.float32)
        denom = np.sum(normal * safe_normalize(res_safe), axis=-1)
        len_plane = np.where(zero_r[:, 0], 0.0, cos_nop / denom).astype(np.float32)
        len_signed = lengths0[sequence].ravel()
        lengths_seq = np.where(len_signed != 0.0, len_signed, len_plane).astype(
            np.float32
        )
        xyz_next = xyz_seq + lengths_seq[:, None] * safe_normalize(residuals)
        xyz_next = np.where(padded, xyz_seq, xyz_next).astype(np.float32)
        res_list.append(residuals.copy())
        len_list.append(lengths_seq.copy())
        xyz_seq_in = xyz_next
    res_all = np.stack(res_list)
    len_all = np.stack(len_list)
    xyz = xyz[:-1]
    reactions = np.zeros((n_nodes, 3), np.float32)
    reactions[seq[-1], :] = res_all[-1]
    forces = np.where(forces0 != 0.0, forces0, 0.0).astype(np.float32)
    res_trail = res_all[:-1].reshape(-1, 3)
    len_trail = len_all[:-1].reshape(-1, 1)
    trail_forces = np.linalg.norm(res_trail, axis=-1, keepdims=True).astype(
        np.float32
    ) * np.where(len_trail < 0.0, -1.0, 1.0).astype(np.float32)
    forces[seqe.ravel(), :] = trail_forces
    lengths = np.linalg.norm(C @ xyz, axis=-1, keepdims=True).astype(np.float32)
    return xyz, reactions, lengths, loads, forces


N_CORES = 8

# Filled after every device run: the BassKernelResults (exec_time_ns is
# populated when tracing is enabled, e.g. BASS_TRACE=1).
LAST_RUN = None


def kernel(**inputs):
    global LAST_RUN
    if not _structure_ok(inputs):
        return _numpy_reference(**inputs)
    staged = _stage_inputs(inputs)
    nc = _build_program()
    in_maps = [dict(staged) for _ in range(N_CORES)]
    res = run_bass_kernel_spmd(nc, in_maps, core_ids=list(range(N_CORES)))
    LAST_RUN = res
    dev = res.results[0]
    return _assemble_outputs(dev, inputs)


if __name__ == "__main__":
    sys.path.insert(0, "/root/problem")
    import reference

    inputs = {k: np.asarray(v) for k, v in reference.setup_inputs().items()}
    outs = kernel(**inputs)
    print([o.shape for o in outs])
